# revision 8
# baseline (speedup 1.0000x reference)
"""Trainium2 Bass kernel for FOAM embedding (GNN message passing).

Strategy (8 NeuronCores, SPMD, no collectives):
  - Edges sorted by edge_src; host partitions nodes into 8 contiguous
    ranges with balanced edge counts; packs each core's edges into
    exact-128-edge blocks with 8 node slots (slot 7 = split head whose
    tail continues in the next block's slot 0).
  - Device scatter: per block one matmul PSUM[basis, (slot,m)] =
    dij^T @ S with dij [128e, 128b] and S [128e, 72] = Y (x) onehot.
    S is slot-major so the slot7->slot0 merge is a 2x-mode DVE add.
  - Dij is a hybrid: some block groups ship dense bf16 from host, the
    rest are built on device (DVE/Pool) as senc (x) rb outer products
    from factored 24-col inputs, cutting HBM traffic.
  - Phase 3 per 14-block tile per l: x/y matmuls vs wx/wy, y staged to
    SBUF (Act), x*y on DVE, m-adds on DVE, outputs per chunk via DMA.
  - Scatter and phase-3 are interleaved on the PE queue so LDWEIGHTS
    stays hidden and the PE p-state stays at 2.4 GHz.
"""

import os
import sys

import numpy as np

for _p in ("/opt/trn_rl_repo", "/root/.axon_site/_ro/trn_rl_repo"):
    if os.path.isdir(_p) and _p not in sys.path:
        sys.path.insert(0, _p)

import ml_dtypes  # noqa: E402

# ---------------- problem constants (hardcoded per spec) ----------------
N_RADIAL = 8
N_SPEC = 16
ZMAX = 64
CUTOFF = 5.0
NCHAN = 128
NB = N_RADIAL * N_SPEC  # 128 basis
M9 = 9                  # real SH components up to l=2

NCORES = 8
P = 128                 # edges per block == partitions
NSLOT = 8               # 7 completed-node slots + 1 split-head slot
SC = NSLOT * M9         # 72 S columns per block (slot-major: s*9+m)
TBLK = 14               # blocks per phase-3 tile (5m*14*7 = 490 <= 512)
CH = 42                 # blocks per chunk (6 groups of 7 = 3 p3 tiles)
PSG = 7                 # blocks per scatter PSUM bank (7*72 = 504)

# per-chunk group assignment: which of the 6 PSG groups have their dij
# built on device (engine) vs shipped dense from host. Builds first,
# ships last (so the dense-dij DMA is one contiguous run per chunk).
GASSIGN = ("dve", "pool", "dve", "pool", "ship", "ship")
# engine that drains each scatter PSUM supertile (2 groups) to SBUF
RHO_COPY = ("act", "act", "act")  # 3 supertiles per chunk

BF16 = ml_dtypes.bfloat16

_COMPILED = {}
TRACE = False          # set True to capture an NTFF profile
LAST_RESULT = None     # BassKernelResults of the last kernel() call

_S3, _S5, _S15 = 3.0 ** 0.5, 5.0 ** 0.5, 15.0 ** 0.5
KM = np.array([1.0, _S3, _S3, _S3, _S15, _S15,
               0.5 * _S5, _S15, 0.5 * _S15], np.float32)


# ======================= host-side preprocessing =======================

def _partition_cores(edge_src, n_nodes):
    """Split nodes into NCORES contiguous ranges with ~equal edges."""
    es = np.asarray(edge_src, dtype=np.int64)
    E = es.shape[0]
    splits = [0]
    for c in range(1, NCORES):
        n = int(es[min((c * E) // NCORES, E - 1)])
        n = max(n, splits[-1])
        splits.append(n)
    splits.append(n_nodes)
    return splits


def _pack_core(deg, first_edge, nlo, nhi):
    """Pack nodes [nlo, nhi) into exact-128-edge blocks.

    Returns (blocks, slot_node) where blocks is a list of
    (e_start, n_edges, cnts[8]) and slot_node is [nblk, 8] node ids
    for completed slots (slots 0..6; -1 elsewhere).
    """
    blocks = []
    slot_nodes = []
    n = nlo
    carry = None  # (node, e_start, cnt) continuation -> slot 0
    while n < nhi or carry is not None:
        cnts = [0] * NSLOT
        snode = [-1] * NSLOT
        cap = P
        e_start = None
        si = 0
        if carry is not None:
            node, es0, cnt = carry
            assert cnt <= cap, f"node {node} degree too large"
            e_start = es0
            cnts[0] = cnt
            snode[0] = node
            cap -= cnt
            si = 1
            carry = None
        while n < nhi and si < NSLOT - 1:
            d = int(deg[n])
            if d > cap:
                break
            if e_start is None:
                e_start = int(first_edge[n])
            cnts[si] = d
            snode[si] = n
            cap -= d
            si += 1
            n += 1
        if cap > 0 and n < nhi:
            # split head into slot 7 (tail continues next block slot 0)
            d = int(deg[n])
            take = min(d, cap)
            if e_start is None:
                e_start = int(first_edge[n])
            cnts[NSLOT - 1] = take
            cap -= take
            carry = (n, int(first_edge[n]) + take, d - take)
            n += 1
        if e_start is None:
            e_start = int(first_edge[min(n, nhi - 1)])
        blocks.append((e_start, P - cap, cnts))
        slot_nodes.append(snode)
    return blocks, np.asarray(slot_nodes, np.int64)


def _chunk_plan(B):
    """Chunk/group layout: list of chunk sizes and per-chunk built/ship
    block index lists (relative to chunk start)."""
    chs = []
    r = B
    while r > 0:
        c = min(CH, r)
        chs.append(c)
        r -= c
    plans = []
    for ch in chs:
        ngrp = (ch + PSG - 1) // PSG
        built, ship = [], []
        beng = []
        for g in range(ngrp):
            k0, k1 = g * PSG, min((g + 1) * PSG, ch)
            a = GASSIGN[g % len(GASSIGN)]
            if a == "ship":
                ship.extend(range(k0, k1))
            else:
                built.extend(range(k0, k1))
                beng.append((a, k0, k1))
        plans.append((ch, built, ship, beng))
    return chs, plans


def _build_core_inputs(blocks, B, ysw_e, senc_e, rb_e, dij_e, plans):
    """Build device DRAM arrays for one core.

    Returns s [128, B*72] bf16, srb [128, nbuilt*24] bf16,
    dijd [128, nship*128] bf16.
    """
    nb = len(blocks)
    eb = np.array([b[0] for b in blocks], np.int64)
    ne = np.array([b[1] for b in blocks], np.int64)
    cnts = np.array([b[2] for b in blocks], np.int64)  # [nb, 8]

    blk_of = np.repeat(np.arange(nb), ne)              # per packed edge
    row_of = np.arange(ne.sum()) - np.repeat(np.cumsum(ne) - ne, ne)
    edge_of = np.repeat(eb, ne) + row_of
    slot_of = np.concatenate([
        np.repeat(np.arange(NSLOT), cnts[k]) for k in range(nb)
    ]) if nb else np.zeros(0, np.int64)

    # S: slot-major [B, P, 8, 9]
    S = np.zeros((B, P, NSLOT, M9), np.float32)
    S[blk_of, row_of, slot_of, :] = ysw_e[edge_of]
    s = np.ascontiguousarray(
        S.transpose(1, 0, 2, 3)).reshape(P, B * SC).astype(BF16)

    # factored senc/rb [B, P, 16+8] and dense dij [B, P, 128]
    SRB = np.zeros((B, P, N_SPEC + N_RADIAL), np.float32)
    SRB[blk_of, row_of, :N_SPEC] = senc_e[edge_of]
    SRB[blk_of, row_of, N_SPEC:] = rb_e[edge_of]

    built_idx, ship_idx = [], []
    c0 = 0
    for ch, built, ship, _ in plans:
        built_idx.extend(c0 + k for k in built)
        ship_idx.extend(c0 + k for k in ship)
        c0 += ch
    built_idx = np.array(built_idx, np.int64)
    ship_idx = np.array(ship_idx, np.int64)

    srb = np.ascontiguousarray(
        SRB[built_idx].transpose(1, 0, 2)
    ).reshape(P, -1).astype(BF16)

    if len(ship_idx):
        D = np.zeros((len(ship_idx), P, NB), np.float32)
        # fill dense dij rows for shipped blocks
        mask = np.isin(blk_of, ship_idx)
        # map global block idx -> position in ship_idx
        pos = np.full(B, -1, np.int64)
        pos[ship_idx] = np.arange(len(ship_idx))
        D[pos[blk_of[mask]], row_of[mask], :] = dij_e[edge_of[mask]]
        dijd = np.ascontiguousarray(
            D.transpose(1, 0, 2)).reshape(P, -1).astype(BF16)
    else:
        dijd = np.zeros((P, 0), BF16)
    return s, srb, dijd


def _perm_w(W):
    """Permute Dense weight rows from rs-order (r*16+s) to (s*8+r)."""
    W = np.asarray(W, np.float32)
    return np.ascontiguousarray(
        W.reshape(N_RADIAL, N_SPEC, -1).transpose(1, 0, 2).reshape(NB, -1)
    )


# ========================= device program =========================

def _build_program(B):
    import concourse.bacc as bacc
    import concourse.mybir as mybir
    import concourse.tile as tile
    from concourse.alu_op_type import AluOpType as alu

    fp32 = mybir.dt.float32
    bf16 = mybir.dt.bfloat16

    assert B % TBLK == 0
    chs, plans = _chunk_plan(B)
    cstart = np.cumsum([0] + chs).tolist()
    # running offsets into compacted srb / dijd dram tensors (in blocks)
    srb_off = [0]
    dij_off = [0]
    for ch, built, ship, _ in plans:
        srb_off.append(srb_off[-1] + len(built))
        dij_off.append(dij_off[-1] + len(ship))
    nbuilt_tot = srb_off[-1]
    nship_tot = dij_off[-1]
    B7 = B * (NSLOT - 1)  # output slots per l

    nc = bacc.Bacc("TRN2", target_bir_lowering=False, debug=False,
                   num_devices=NCORES)

    s_d = nc.dram_tensor("s", [P, B * SC], bf16, kind="ExternalInput")
    srb_d = nc.dram_tensor("srb", [P, max(nbuilt_tot, 1) * 24], bf16,
                           kind="ExternalInput")
    dijd_d = nc.dram_tensor("dijd", [P, max(nship_tot, 1) * NB], bf16,
                            kind="ExternalInput")
    wx_d = nc.dram_tensor("wx", [P, 3 * NCHAN], bf16, kind="ExternalInput")
    wy_d = nc.dram_tensor("wy", [P, 3 * NCHAN], bf16, kind="ExternalInput")
    r0_d = nc.dram_tensor("rhoi0", [P, B7], bf16, kind="ExternalOutput")
    xy_d = nc.dram_tensor("xy", [P, 3 * B7], bf16, kind="ExternalOutput")

    with tile.TileContext(nc) as tc:
        with (
            tc.tile_pool(name="const", bufs=1) as cpool,
            tc.tile_pool(name="chunk", bufs=3) as ckpool,
            tc.tile_pool(name="big", bufs=3) as bigpool,
            tc.tile_pool(name="work", bufs=2) as wkpool,
            tc.tile_pool(name="out", bufs=3) as opool,
            tc.tile_pool(name="ps_sc", bufs=2, space="PSUM") as pssc,
            tc.tile_pool(name="ps_x", bufs=2, space="PSUM") as psx,
            tc.tile_pool(name="ps_y", bufs=2, space="PSUM") as psy,
        ):
            wx = cpool.tile([P, 3 * NCHAN], bf16, tag="wx")
            wy = cpool.tile([P, 3 * NCHAN], bf16, tag="wy")
            nc.sync.dma_start(out=wx[:], in_=wx_d[:])
            nc.sync.dma_start(out=wy[:], in_=wy_d[:])

            # HAM warm-up primer: back-to-back dummy matmuls while the
            # first chunk DMAs land, so the PE clock is ramping before
            # real work starts.
            dum = cpool.tile([P, NCHAN], bf16, tag="dum")
            nc.vector.memset(dum[:], 0.0)
            psdum = psx.tile([P, 512], fp32, tag="xp")
            for _ in range(28):
                nc.tensor.matmul(out=psdum[:, 0:NCHAN], lhsT=dum[:],
                                 rhs=dum[:], start=True, stop=True)

            rtiles = {}   # ci -> rhoi sbuf tile
            dtiles = {}   # ci -> (dij, s, srb)

            def dma_part(ci):
                ch, built, ship, beng = plans[ci]
                c0 = cstart[ci]
                dij = ckpool.tile([P, CH * NB], bf16, tag="dij")
                s = ckpool.tile([P, CH * SC], bf16, tag="s")
                srb = ckpool.tile([P, CH * 24], bf16, tag="srb")
                dtiles[ci] = (dij, s, srb)
                nc.sync.dma_start(
                    out=s[:, 0:ch * SC],
                    in_=s_d[:, c0 * SC:(c0 + ch) * SC])
                nbu = len(built)
                if nbu:
                    o = srb_off[ci]
                    nc.sync.dma_start(
                        out=srb[:, 0:nbu * 24],
                        in_=srb_d[:, o * 24:(o + nbu) * 24])
                nsh = len(ship)
                if nsh:
                    o = dij_off[ci]
                    k0 = ship[0]
                    nc.sync.dma_start(
                        out=dij[:, k0 * NB:(k0 + nsh) * NB],
                        in_=dijd_d[:, o * NB:(o + nsh) * NB])

            def build_part(ci):
                # device-side dij = senc (x) rb for built groups
                ch, built, ship, beng = plans[ci]
                dij, s, srb = dtiles[ci]
                # srb is compact over built blocks: block built[i] is at
                # compact index i. Groups are contiguous runs of built.
                cpos = {k: i for i, k in enumerate(built)}
                dv = dij[:].rearrange("p (k s r) -> p k s r",
                                      s=N_SPEC, r=N_RADIAL)
                sv = srb[:].rearrange("p (k c) -> p k c", c=24)
                for eng, k0, k1 in beng:
                    n = k1 - k0
                    i0 = cpos[k0]
                    senc = sv[:, i0:i0 + n, 0:N_SPEC]
                    rb = sv[:, i0:i0 + n, N_SPEC:24]
                    out = dv[:, k0:k1]
                    in0 = senc.unsqueeze(3).broadcast_to(
                        [P, n, N_SPEC, N_RADIAL])
                    in1 = rb.unsqueeze(2).broadcast_to(
                        [P, n, N_SPEC, N_RADIAL])
                    e = nc.vector if eng == "dve" else nc.gpsimd
                    e.tensor_tensor(out=out, in0=in0, in1=in1,
                                    op=alu.mult)

            def scatter_part(ci):
                ch = chs[ci]
                dij, s, srb = dtiles[ci]

                rhoi = bigpool.tile([P, CH * SC], bf16, tag="rhoi")
                rtiles[ci] = rhoi
                rv = rhoi[:].rearrange("p (k sl m) -> p k sl m",
                                       sl=NSLOT, m=M9)

                nsup = (ch + 2 * PSG - 1) // (2 * PSG)
                for sup in range(nsup):
                    # supertile: 2 PSUM banks, groups at col 0 and 512
                    pst = pssc.tile([P, 1024], fp32, tag="psc")
                    kbase = sup * 2 * PSG
                    nblk = min(2 * PSG, ch - kbase)
                    for j in range(nblk):
                        k = kbase + j
                        colb = (j // PSG) * 512 + (j % PSG) * SC
                        nc.tensor.matmul(
                            out=pst[:, colb:colb + SC],
                            lhsT=dij[:, k * NB:(k + 1) * NB],
                            rhs=s[:, k * SC:(k + 1) * SC],
                            start=True, stop=True,
                        )
                    # drain supertile to SBUF (bf16); nblk is always 14
                    # (B is a multiple of TBLK), so one 4D copy covers
                    # both banks and skips the 8-col bank padding.
                    assert nblk == 2 * PSG
                    eng = RHO_COPY[sup % len(RHO_COPY)]
                    src = pst[:].rearrange("p (g q) -> p g q", g=2)[
                        :, :, 0:PSG * SC].rearrange(
                        "p g (k c) -> p g k c", c=SC)
                    dst = rhoi[:, kbase * SC:(kbase + nblk) * SC].rearrange(
                        "p (g k c) -> p g k c", g=2, c=SC)
                    if eng == "act":
                        nc.scalar.copy(out=dst, in_=src)
                    else:
                        nc.vector.tensor_copy(out=dst, in_=src)
                    # merge split-node partials slot7[k-1] -> slot0[k]
                    # (2x-mode DVE adds: last dim m is contiguous)
                    k0 = kbase if kbase > 0 else 1
                    k1 = kbase + nblk
                    if k1 > k0:
                        nc.vector.tensor_tensor(
                            out=rv[:, k0:k1, 0, :],
                            in0=rv[:, k0:k1, 0, :],
                            in1=rv[:, k0 - 1:k1 - 1, 7, :],
                            op=alu.add,
                        )
                    if sup == 0 and ci > 0:
                        pch = chs[ci - 1]
                        prv = rtiles[ci - 1][:].rearrange(
                            "p (k sl m) -> p k sl m", sl=NSLOT, m=M9)
                        nc.vector.tensor_tensor(
                            out=rv[:, 0:1, 0, :],
                            in0=rv[:, 0:1, 0, :],
                            in1=prv[:, pch - 1:pch, 7, :],
                            op=alu.add,
                        )

            def r0_part(ci):
                ch = chs[ci]
                c0 = cstart[ci]
                rv = rtiles[ci][:].rearrange("p (k sl m) -> p k sl m",
                                             sl=NSLOT, m=M9)
                r0t = opool.tile([P, CH * 7], bf16, tag="r0t")
                nc.gpsimd.tensor_copy(
                    out=r0t[:, 0:ch * 7].rearrange("p (k s) -> p k s", s=7),
                    in_=rv[:, 0:ch, 0:7, 0],
                )
                nc.sync.dma_start(out=r0_d[:, c0 * 7:(c0 + ch) * 7],
                                  in_=r0t[:, 0:ch * 7])

            def phase3_tile(ci, t):
                ch = chs[ci]
                rv = rtiles[ci][:].rearrange("p (k sl m) -> p k sl m",
                                             sl=NSLOT, m=M9)
                kk = t * TBLK
                nk = min(TBLK, ch - kk)
                ns = nk * 7
                ol = _ols[ci][:].rearrange("p (l b) -> p l b", l=3)
                for l in range(3):
                    mg = 2 * l + 1
                    m0 = l * l
                    wxl = wx[:, l * NCHAN:(l + 1) * NCHAN]
                    wyl = wy[:, l * NCHAN:(l + 1) * NCHAN]
                    xp = psx.tile([P, 512], fp32, tag="xp")
                    yp = psy.tile([P, 512], fp32, tag="yp")
                    for mi in range(mg):
                        mov = rv[:, kk:kk + nk, 0:7, m0 + mi]
                        nc.tensor.matmul(
                            out=xp[:, mi * ns:(mi + 1) * ns],
                            lhsT=wxl, rhs=mov, start=True, stop=True)
                    for mi in range(mg):
                        mov = rv[:, kk:kk + nk, 0:7, m0 + mi]
                        nc.tensor.matmul(
                            out=yp[:, mi * ns:(mi + 1) * ns],
                            lhsT=wyl, rhs=mov, start=True, stop=True)
                    ysb = wkpool.tile([P, 512], bf16, tag="ysb")
                    nc.scalar.copy(out=ysb[:, 0:mg * ns],
                                   in_=yp[:, 0:mg * ns])
                    od = ol[:, l, t * 98:t * 98 + ns]
                    if l == 0:
                        nc.vector.tensor_tensor(
                            out=od, in0=xp[:, 0:ns], in1=ysb[:, 0:ns],
                            op=alu.mult)
                        continue
                    pl = wkpool.tile([P, 512], bf16, tag="pl")
                    nc.vector.tensor_tensor(
                        out=pl[:, 0:mg * ns],
                        in0=xp[:, 0:mg * ns], in1=ysb[:, 0:mg * ns],
                        op=alu.mult)
                    pv = pl[:, 0:mg * ns].rearrange("p (m s) -> p m s",
                                                    s=ns)
                    if l == 1:
                        tmp = wkpool.tile([P, 128], bf16, tag="tmp1")
                        nc.vector.tensor_tensor(
                            out=tmp[:, 0:ns], in0=pv[:, 0, :],
                            in1=pv[:, 1, :], op=alu.add)
                        nc.vector.tensor_tensor(
                            out=od, in0=tmp[:, 0:ns],
                            in1=pv[:, 2, :], op=alu.add)
                    else:
                        ta = wkpool.tile([P, 128], bf16, tag="tmp2a")
                        tb = wkpool.tile([P, 128], bf16, tag="tmp2b")
                        nc.vector.tensor_tensor(
                            out=ta[:, 0:ns], in0=pv[:, 0, :],
                            in1=pv[:, 1, :], op=alu.add)
                        nc.vector.tensor_tensor(
                            out=tb[:, 0:ns], in0=pv[:, 2, :],
                            in1=pv[:, 3, :], op=alu.add)
                        nc.vector.tensor_tensor(
                            out=ta[:, 0:ns], in0=ta[:, 0:ns],
                            in1=tb[:, 0:ns], op=alu.add)
                        nc.vector.tensor_tensor(
                            out=od, in0=ta[:, 0:ns],
                            in1=pv[:, 4, :], op=alu.add)

            _ols = {}

            def phase3_open(ci):
                ol_t = opool.tile([P, 3 * CH * 7], bf16, tag="ol")
                _ols[ci] = ol_t

            def phase3_close(ci):
                ch = chs[ci]
                c0 = cstart[ci]
                ol = _ols.pop(ci)
                olv = ol[:].rearrange("p (l b) -> p l b", l=3)
                nc.sync.dma_start(
                    out=xy_d[:].rearrange("p (l b) -> p l b", l=3)[
                        :, :, c0 * 7:(c0 + ch) * 7],
                    in_=olv[:, :, 0:ch * 7])

            # software pipeline: input DMA two chunks ahead; dij builds
            # one chunk ahead of their scatter; phase-3 one chunk behind
            # scatter and emitted BEFORE it (its deps are already met,
            # avoiding head-of-line blocking on the in-order PE queue).
            nchunk = len(chs)
            for ci in range(nchunk + 2):
                if ci < nchunk:
                    dma_part(ci)
                cs = ci - 1           # scatter chunk
                cp = ci - 2           # phase-3 chunk
                if 0 <= cs < nchunk:
                    build_part(cs)
                if 0 <= cp < nchunk:
                    phase3_open(cp)
                    ntile = (chs[cp] + TBLK - 1) // TBLK
                    for t in range(ntile):
                        phase3_tile(cp, t)
                    phase3_close(cp)
                if 0 <= cs < nchunk:
                    scatter_part(cs)
                    r0_part(cs)

    nc.finalize()
    return nc


# ============================ entry point ============================

def kernel(**inputs):
    from concourse.bass_utils import run_bass_kernel_spmd

    dist = np.asarray(inputs["distances"], np.float32)
    vec = np.asarray(inputs["vec"], np.float32)
    switch = np.asarray(inputs["switch"], np.float32)
    st = np.asarray(inputs["species_table"], np.float32)
    species = np.asarray(inputs["species"], np.int64)
    esrc = np.asarray(inputs["edge_src"], np.int64)
    edst = np.asarray(inputs["edge_dst"], np.int64)
    N_NODES = species.shape[0]
    E = esrc.shape[0]

    deg = np.bincount(esrc, minlength=N_NODES)
    assert deg.max() <= P, "node degree exceeds 128"
    first_edge = np.searchsorted(esrc, np.arange(N_NODES + 1), side="left")
    splits = _partition_cores(esrc, N_NODES)

    # per-edge factors
    nvec = np.arange(1, N_RADIAL + 1, dtype=np.float32)
    rb_e = (np.sqrt(2.0 / CUTOFF) * np.sin(nvec[None, :] * (np.pi / CUTOFF)
                                           * dist[:, None]) / dist[:, None]
            * switch[:, None]).astype(np.float32)           # [E, 8]
    senc_e = st[species[edst]]                              # [E, 16]
    u = vec / dist[:, None]
    x, y, z = u[:, 0], u[:, 1], u[:, 2]
    ysw_e = (np.stack([
        np.ones_like(x), x, y, z, x * y, y * z,
        3.0 * z * z - 1.0, x * z, x * x - y * y,
    ], axis=-1) * KM[None, :]).astype(np.float32)

    cores = []
    maxb = 0
    for c in range(NCORES):
        blocks, slot_node = _pack_core(deg, first_edge,
                                       splits[c], splits[c + 1])
        cores.append((blocks, slot_node))
        maxb = max(maxb, len(blocks))
    B = ((maxb + TBLK - 1) // TBLK) * TBLK
    B7 = B * (NSLOT - 1)
    chs, plans = _chunk_plan(B)

    # dij for shipped blocks: match device build numerics (bf16 inputs)
    dij_e = (senc_e.astype(BF16).astype(np.float32)[:, :, None]
             * rb_e.astype(BF16).astype(np.float32)[:, None, :]
             ).reshape(E, NB)

    wx = np.empty((P, 3 * NCHAN), np.float32)
    wy = np.empty((P, 3 * NCHAN), np.float32)
    for l, key in enumerate(("W0", "W1", "W2")):
        Wp = _perm_w(inputs[key])
        wx[:, l * NCHAN:(l + 1) * NCHAN] = Wp[:, :NCHAN]
        wy[:, l * NCHAN:(l + 1) * NCHAN] = (
            Wp[:, NCHAN:] / np.sqrt(2 * l + 1.0))
    wx = wx.astype(BF16)
    wy = wy.astype(BF16)

    in_maps = []
    for c in range(NCORES):
        blocks, _ = cores[c]
        s, srb, dijd = _build_core_inputs(
            blocks, B, ysw_e, senc_e, rb_e, dij_e, plans)
        if srb.shape[1] == 0:
            srb = np.zeros((P, 24), BF16)
        if dijd.shape[1] == 0:
            dijd = np.zeros((P, NB), BF16)
        in_maps.append({"s": s, "srb": srb, "dijd": dijd,
                        "wx": wx, "wy": wy})

    if B not in _COMPILED:
        _COMPILED[B] = _build_program(B)
    nc = _COMPILED[B]

    res = run_bass_kernel_spmd(nc, in_maps, list(range(NCORES)),
                               trace=TRACE)
    global LAST_RESULT
    LAST_RESULT = res

    # ---------------- host assembly ----------------
    out = np.zeros((N_NODES, N_SPEC + NB + 3 * NCHAN), np.float32)
    out[:, :N_SPEC] = st[species]

    # device basis row of original index rs = r*16+s is dev = s*8+r
    r = np.arange(NB) // N_SPEC
    sidx = np.arange(NB) % N_SPEC
    dev_of_rs = sidx * N_RADIAL + r

    for c in range(NCORES):
        _, slot_node = cores[c]
        sn = np.full((B, NSLOT - 1), -1, np.int64)
        sn[:slot_node.shape[0]] = slot_node[:, :NSLOT - 1]
        sn = sn.reshape(-1)
        valid = sn >= 0
        nodes = sn[valid]
        slots = np.nonzero(valid)[0]
        r0 = np.asarray(res.results[c]["rhoi0"], np.float32)  # [128, B7]
        xy = np.asarray(res.results[c]["xy"], np.float32)     # [128, 3*B7]
        out[nodes, N_SPEC:N_SPEC + NB] = r0[dev_of_rs][:, slots].T
        for l in range(3):
            out[nodes,
                N_SPEC + NB + l * NCHAN:N_SPEC + NB + (l + 1) * NCHAN] = (
                xy[:, l * B7 + slots].T)
    return out
